# revision 1
# baseline (speedup 1.0000x reference)
"""BERT (12-layer, B=16, S=512, D=768) forward pass on 8 Trainium2 NeuronCores.

Strategy: data-parallel over batch — each of the 8 cores runs the full
12-layer encoder on 2 sequences (1024 tokens). No collectives.

Layouts (per core):
  - Residual stream x: fp32, natural layout [128 partitions (token%128), 8 s-tiles, 768]
  - LN outputs transposed to [d, token] (PE transpose) so QKV/FFN matmuls
    contract over d on partitions; LN gain/bias applied during the
    transposed copy via ScalarE per-partition scale/bias.
  - Attention computed per head with scoresT layout [t, s]: softmax without
    max subtraction (scores bounded ~±2 after 1/8 scaling), probsT consumed
    directly as the stationary operand of attn@V, producing oT [e, s] which
    feeds the output projection without further transposes.
  - Row sums of exp via an appended ones-column on V (one extra matmul col);
    normalization applied to oT with a PE-broadcast reciprocal row.
  - All matmuls bf16 with fp32 PSUM accumulation; residual stream fp32.
"""

import os
import numpy as np
import ml_dtypes

import concourse.bacc as bacc
import concourse.bass as bass
import concourse.mybir as mybir
import concourse.tile as tile
from concourse import bass_utils
from concourse.bass import IndirectOffsetOnAxis
from concourse.masks import make_identity

BF16 = ml_dtypes.bfloat16
F32 = mybir.dt.float32
B16 = mybir.dt.bfloat16

B, S, D, H, DH, L, V = 16, 512, 768, 12, 64, 12, 30522
FF = 4 * D
EPS = 1e-5
NCORES = 8
BL = B // NCORES          # sequences per core
NT = BL * S // 128        # 8 token tiles per core
ST = S // 128             # 4 s-tiles per sequence
DC = D // 128             # 6 d-chunks
FT = FF // 128            # 24 ff tiles
NSL = 384                 # free-dim split of D (768 = 2*384), fits PSUM bank
AFT = DH + 1              # 65: v columns + ones column



def _offsets():
    wb, fb = {}, {}
    o = 0
    for nm, sz in (("wq", L * 128 * DC * D), ("wk", L * 128 * DC * D),
                   ("wv", L * 128 * DC * D), ("wo", L * 128 * DC * D),
                   ("w1p", L * FT * 128 * D), ("w2p", L * FT * 128 * D),
                   ("brows", L * 2 * D), ("lnemb", 2 * D), ("lnf", 2 * D)):
        wb[nm] = o; o += sz
    wb_total = o
    o = 0
    for nm, sz in (("semb", 128 * ST * D), ("lng", L * 128 * 2 * DC),
                   ("lnb", L * 128 * 2 * DC), ("b1t", L * 128 * FT)):
        fb[nm] = o; o += sz
    return wb, wb_total, fb, o


WB_OFF, WB_TOTAL, FB_OFF, FB_TOTAL = _offsets()

Act = mybir.ActivationFunctionType
Alu = mybir.AluOpType


def build_nc(n_layers=L):
    nc = bacc.Bacc("TRN2", target_bir_lowering=False, debug=False,
                   num_devices=NCORES)
    dt = nc.dram_tensor
    t_idx = dt("idxw", [128, NT], mybir.dt.int32, kind="ExternalInput").ap()
    t_emb = dt("temb", [V, D], B16, kind="ExternalInput").ap()
    t_wb = dt("wb", [WB_TOTAL], B16, kind="ExternalInput").ap()
    t_fb = dt("fb", [FB_TOTAL], F32, kind="ExternalInput").ap()

    def wslice(off, n, p=128):
        return t_wb[off:off + n].rearrange("(p m) -> p m", p=p)

    def fslice(off, n, p=128):
        return t_fb[off:off + n].rearrange("(p m) -> p m", p=p)

    t_semb = fslice(FB_OFF["semb"], 128 * ST * D)
    t_lnemb = wslice(WB_OFF["lnemb"], 2 * D, p=1)
    t_lnf = wslice(WB_OFF["lnf"], 2 * D, p=1)
    t_wq = [wslice(WB_OFF["wq"] + l * 128 * DC * D, 128 * DC * D) for l in range(n_layers)]
    t_wk = [wslice(WB_OFF["wk"] + l * 128 * DC * D, 128 * DC * D) for l in range(n_layers)]
    t_wv = [wslice(WB_OFF["wv"] + l * 128 * DC * D, 128 * DC * D) for l in range(n_layers)]
    t_wo = [wslice(WB_OFF["wo"] + l * 128 * DC * D, 128 * DC * D) for l in range(n_layers)]
    t_w1 = [[wslice(WB_OFF["w1p"] + (l * FT + ft) * 128 * D, 128 * D)
             for ft in range(FT)] for l in range(n_layers)]
    t_w2 = [[wslice(WB_OFF["w2p"] + (l * FT + ft) * 128 * D, 128 * D)
             for ft in range(FT)] for l in range(n_layers)]
    t_lng = [fslice(FB_OFF["lng"] + l * 128 * 2 * DC, 128 * 2 * DC) for l in range(n_layers)]
    t_lnb = [fslice(FB_OFF["lnb"] + l * 128 * 2 * DC, 128 * 2 * DC) for l in range(n_layers)]
    t_b1 = [fslice(FB_OFF["b1t"] + l * 128 * FT, 128 * FT) for l in range(n_layers)]
    t_br = [wslice(WB_OFF["brows"] + l * 2 * D, 2 * D, p=1) for l in range(n_layers)]
    t_out = dt("xo", [NT, 128, D], F32, kind="ExternalOutput").ap()

    with tile.TileContext(nc) as tc:
        _body(tc, n_layers, t_idx, t_emb, t_semb, t_lnemb, t_lnf,
              t_wq, t_wk, t_wv, t_wo, t_w1, t_w2, t_lng, t_lnb, t_b1,
              t_br, t_out)
    nc.compile()
    return nc


def _body(tc, n_layers, t_idx, t_emb, t_semb, t_lnemb, t_lnf,
          t_wq, t_wk, t_wv, t_wo, t_w1, t_w2, t_lng, t_lnb, t_b1,
          t_br, t_out):
    nc = tc.nc
    from contextlib import ExitStack
    ctx = ExitStack()
    with ctx:
        pconst = ctx.enter_context(tc.tile_pool(name="pconst", bufs=1))
        pgb = ctx.enter_context(tc.tile_pool(name="pgb", bufs=2))
        px = ctx.enter_context(tc.tile_pool(name="px", bufs=1))
        pw = ctx.enter_context(tc.tile_pool(name="pw", bufs=1))
        psm = ctx.enter_context(tc.tile_pool(name="psm", bufs=2))
        pw1 = ctx.enter_context(tc.tile_pool(name="pw1", bufs=3))
        pw2 = ctx.enter_context(tc.tile_pool(name="pw2", bufs=3))
        pact = ctx.enter_context(tc.tile_pool(name="pact", bufs=1))
        pprob = ctx.enter_context(tc.tile_pool(name="pprob", bufs=3))
        pyn = ctx.enter_context(tc.tile_pool(name="pyn", bufs=5))
        pstat = ctx.enter_context(tc.tile_pool(name="pstat", bufs=8))
        pbc = ctx.enter_context(tc.tile_pool(name="pbc", bufs=2))
        pfin = ctx.enter_context(tc.tile_pool(name="pfin", bufs=1))
        pps = ctx.enter_context(tc.tile_pool(name="pps", bufs=8, space="PSUM"))

        def ps_tile(shape, dtype=F32):
            return pps.tile(shape, dtype, tag="ps", name="ps")

        # ---- constants ----
        ident = pconst.tile([128, 128], B16, tag="ident", name="ident")
        make_identity(nc, ident[:])
        ones = pconst.tile([1, 128], B16, tag="ones", name="ones")
        nc.vector.memset(ones[:], 1.0)
        epsc = pconst.tile([128, 1], F32, tag="epsc", name="epsc")
        nc.vector.memset(epsc[:], EPS)

        idx_sb = pconst.tile([128, NT], mybir.dt.int32, tag="idx", name="idx")
        nc.sync.dma_start(idx_sb[:], t_idx[:])
        semb_sb = pw.tile([128, ST, D], F32, tag="wq", name="semb")
        nc.sync.dma_start(semb_sb[:], t_semb.rearrange("p (s d) -> p s d", s=ST))
        lnemb_sb = pconst.tile([1, 2 * D], B16, tag="lnemb", name="lnemb")
        nc.sync.dma_start(lnemb_sb[:], t_lnemb)
        lnf_sb = pconst.tile([1, 2 * D], B16, tag="lnf", name="lnf")
        nc.sync.dma_start(lnf_sb[:], t_lnf)

        def bcast_rows(src_row):
            """[1, D] bf16 row -> [128, D] fp32 tile via PE outer product."""
            out = pgb.tile([128, D], F32, tag="gbcast", name="gbcast")
            for sl in range(2):
                ps = ps_tile([128, NSL])
                nc.tensor.matmul(ps[:], ones[:], src_row[:, sl * NSL:(sl + 1) * NSL],
                                 start=True, stop=True)
                nc.scalar.copy(out[:, sl * NSL:(sl + 1) * NSL], ps[:])
            return out

        # ---- embedding ----
        x = px.tile([128, NT, D], F32, tag="x", name="x")
        for i in range(NT):
            tg = pyn.tile([128, D], B16, tag="yn", name="tg")
            nc.gpsimd.indirect_dma_start(
                out=tg[:], out_offset=None, in_=t_emb[:, :],
                in_offset=IndirectOffsetOnAxis(ap=idx_sb[:, i:i + 1], axis=0))
            nc.vector.tensor_add(x[:, i, :], tg[:], semb_sb[:, i % ST, :])

        g_emb = bcast_rows(lnemb_sb[0:1, 0:D])
        b_emb = bcast_rows(lnemb_sb[0:1, D:2 * D])

        def ln_stats(x_slice):
            """Return (mean[128,1], rstd[128,1]) for a [128, D] fp32 slice."""
            st_ = pstat.tile([128, 12], F32, tag="bnst", name="bnst")
            nc.vector.bn_stats(st_[:, 0:6], x_slice[:, 0:NSL])
            nc.vector.bn_stats(st_[:, 6:12], x_slice[:, NSL:2 * NSL])
            ag = pstat.tile([128, 2], F32, tag="bnag", name="bnag")
            nc.vector.bn_aggr(ag[:], st_[:].rearrange("p (c k) -> p c k", k=6))
            sd = pstat.tile([128, 1], F32, tag="sd", name="sd")
            nc.scalar.activation(sd[:], ag[:, 1:2], Act.Sqrt, bias=epsc[:])
            rstd = pstat.tile([128, 1], F32, tag="rstd", name="rstd")
            nc.vector.reciprocal(rstd[:], sd[:])
            return ag, rstd

        for i in range(NT):
            ag, rstd = ln_stats(x[:, i, :])
            nc.vector.tensor_scalar(x[:, i, :], x[:, i, :], ag[:, 0:1], rstd[:],
                                    op0=Alu.subtract, op1=Alu.mult)
            nc.vector.tensor_mul(x[:, i, :], x[:, i, :], g_emb[:])
            nc.vector.tensor_add(x[:, i, :], x[:, i, :], b_emb[:])

        # ---- layers ----
        for l in range(n_layers):
            wq_sb = pw.tile([128, DC, D], B16, tag="wq", name="wq")
            nc.sync.dma_start(wq_sb[:], t_wq[l].rearrange("p (c d) -> p c d", c=DC))
            wk_sb = pw.tile([128, DC, D], B16, tag="wk", name="wk")
            nc.sync.dma_start(wk_sb[:], t_wk[l].rearrange("p (c d) -> p c d", c=DC))
            wv_sb = pw.tile([128, DC, D], B16, tag="wv", name="wv")
            nc.sync.dma_start(wv_sb[:], t_wv[l].rearrange("p (c d) -> p c d", c=DC))
            wo_sb = pw.tile([128, DC, D], B16, tag="wo", name="wo")
            nc.sync.dma_start(wo_sb[:], t_wo[l].rearrange("p (c d) -> p c d", c=DC))
            lng_sb = psm.tile([128, 2 * DC], F32, tag="lng", name="lng")
            nc.sync.dma_start(lng_sb[:], t_lng[l])
            lnb_sb = psm.tile([128, 2 * DC], F32, tag="lnb", name="lnb")
            nc.sync.dma_start(lnb_sb[:], t_lnb[l])
            b1_sb = psm.tile([128, FT], F32, tag="b1", name="b1")
            nc.sync.dma_start(b1_sb[:], t_b1[l])
            br_sb = psm.tile([1, 2 * D], B16, tag="br", name="br")
            nc.sync.dma_start(br_sb[:], t_br[l])

            h2Ts = {}
            def attn_phase(b):
                # ---- LN1 -> transposed hT with gain/bias ----
                hT = pact.tile([128, DC, S], B16, tag="hT", name="hT")
                yns = []
                for st in range(ST):
                    xi = ST * b + st
                    ag, rstd = ln_stats(x[:, xi, :])
                    yn = pyn.tile([128, D], B16, tag="yn", name="yn")
                    nc.vector.tensor_scalar(yn[:], x[:, xi, :], ag[:, 0:1], rstd[:],
                                            op0=Alu.subtract, op1=Alu.mult)
                    yns.append(yn)
                for st in range(ST):
                    yn = yns[st]
                    for dc in range(DC):
                        pt = ps_tile([128, 128], B16)
                        nc.tensor.transpose(pt[:], yn[:, dc * 128:(dc + 1) * 128],
                                            ident[:])
                        nc.scalar.activation(
                            hT[:, dc, st * 128:(st + 1) * 128], pt[:], Act.Identity,
                            bias=lnb_sb[:, dc:dc + 1], scale=lng_sb[:, dc:dc + 1])

                # ---- QKV projections ----
                qT = pact.tile([128, DC, S], B16, tag="qT", name="qT")
                kT = pact.tile([128, DC, S], B16, tag="kT", name="kT")
                for mt in range(DC):
                    for dst, w in ((qT, wq_sb), (kT, wk_sb)):
                        ps = ps_tile([128, S])
                        for dc in range(DC):
                            nc.tensor.matmul(ps[:], w[:, dc, mt * 128:(mt + 1) * 128],
                                             hT[:, dc, :], start=(dc == 0),
                                             stop=(dc == DC - 1))
                        nc.vector.tensor_copy(dst[:, mt, :], ps[:])

                vaug = pact.tile([128, ST, H * AFT], B16, tag="vaug", name="vaug")
                nc.vector.memset(vaug[:], 1.0)
                for tm in range(ST):
                    for sl in range(2):
                        ps = ps_tile([128, NSL])
                        for dc in range(DC):
                            nc.tensor.matmul(ps[:], hT[:, dc, tm * 128:(tm + 1) * 128],
                                             wv_sb[:, dc, sl * NSL:(sl + 1) * NSL],
                                             start=(dc == 0), stop=(dc == DC - 1))
                        out_ap = vaug[:, tm, sl * 6 * AFT:(sl + 1) * 6 * AFT] \
                            .rearrange("p (h w) -> p h w", w=AFT)[:, :, 0:DH]
                        nc.vector.tensor_copy(out_ap, ps[:])

                # ---- attention, software-pipelined across heads ----
                # stage A(h): scoresT + exp; stage B(h): attn@V (+rowsum);
                # stage C(h): normalize into oT.  Emitted as A(h), B(h-2),
                # C(h-3) so PE never waits on the ACT/DVE chains.
                oT = pact.tile([128, DC, S], B16, tag="oT", name="oT")
                probs_t = {}
                ops_t = {}
                def stage_a(h):
                    mt, po = h // 2, (h % 2) * 64
                    probs = pprob.tile([128, ST, S], B16, tag="probs", name="probs")
                    probs_t[h] = probs
                    for tt in range(ST):
                        ps = ps_tile([128, S])
                        nc.tensor.matmul(
                            ps[:], kT[po:po + 64, mt, tt * 128:(tt + 1) * 128],
                            qT[po:po + 64, mt, :], start=True, stop=True,
                            tile_position=(po, 0))
                        nc.scalar.activation(probs[:, tt, :], ps[:], Act.Exp,
                                             scale=float(1.0 / np.sqrt(DH)))
                def stage_b(h):
                    probs = probs_t[h]
                    ops_ = ps_tile([AFT, S])
                    ops_t[h] = ops_
                    for tc_ in range(ST):
                        nc.tensor.matmul(ops_[:], vaug[:, tc_, h * AFT:(h + 1) * AFT],
                                         probs[:, tc_, :], start=(tc_ == 0),
                                         stop=(tc_ == ST - 1))
                def stage_c(h):
                    mt, po = h // 2, (h % 2) * 64
                    ops_ = ops_t.pop(h)
                    probs_t.pop(h)
                    rc = pstat.tile([1, S], F32, tag="rc", name="rc")
                    nc.vector.reciprocal(rc[:], ops_[DH:AFT, :])
                    rcb = pstat.tile([1, S], B16, tag="rcb", name="rcb")
                    nc.vector.tensor_copy(rcb[:], rc[:])
                    bcp = ps_tile([64, S])
                    nc.tensor.matmul(bcp[:], ones[0:1, 0:64], rcb[:],
                                     start=True, stop=True)
                    bcs = pbc.tile([64, S], B16, tag="bcs", name="bcs")
                    if h % 2 == 0:
                        nc.vector.tensor_copy(bcs[:], bcp[:])
                    else:
                        nc.scalar.copy(bcs[:], bcp[:])
                    nc.vector.tensor_mul(oT[po:po + 64, mt, :], ops_[0:DH, :], bcs[:])
                for h in range(H + 3):
                    if h < H:
                        stage_a(h)
                    if 2 <= h < H + 2:
                        stage_b(h - 2)
                    if h >= 3:
                        stage_c(h - 3)

                # ---- output projection + residual ----
                for st in range(ST):
                    xi = ST * b + st
                    for sl in range(2):
                        ps = ps_tile([128, NSL])
                        for ec in range(DC):
                            nc.tensor.matmul(ps[:], oT[:, ec, st * 128:(st + 1) * 128],
                                             wo_sb[:, ec, sl * NSL:(sl + 1) * NSL],
                                             start=(ec == 0), stop=False)
                        nc.tensor.matmul(ps[:], ones[:], br_sb[0:1, sl * NSL:(sl + 1) * NSL],
                                         start=False, stop=True)
                        nc.vector.tensor_add(x[:, xi, sl * NSL:(sl + 1) * NSL],
                                             x[:, xi, sl * NSL:(sl + 1) * NSL], ps[:])

                # ---- LN2 -> h2T ----
                h2T = pact.tile([128, DC, S], B16, tag="h2T", name="h2T", bufs=2)
                yns2 = []
                for st in range(ST):
                    xi = ST * b + st
                    ag, rstd = ln_stats(x[:, xi, :])
                    yn = pyn.tile([128, D], B16, tag="yn", name="yn")
                    nc.vector.tensor_scalar(yn[:], x[:, xi, :], ag[:, 0:1], rstd[:],
                                            op0=Alu.subtract, op1=Alu.mult)
                    yns2.append(yn)
                for st in range(ST):
                    yn = yns2[st]
                    for dc in range(DC):
                        pt = ps_tile([128, 128], B16)
                        nc.tensor.transpose(pt[:], yn[:, dc * 128:(dc + 1) * 128],
                                            ident[:])
                        nc.scalar.activation(
                            h2T[:, dc, st * 128:(st + 1) * 128], pt[:], Act.Identity,
                            bias=lnb_sb[:, DC + dc:DC + dc + 1],
                            scale=lng_sb[:, DC + dc:DC + dc + 1])

                h2Ts[b] = h2T
            def ffn_phase(b):
                h2T = h2Ts.pop(b)
                # ---- FFN up (ffT = relu(W1^T @ h2T + b1)) ----
                ffT = pact.tile([128, FT, S], B16, tag="ffT", name="ffT")
                for ft in range(FT):
                    w1c = pw1.tile([128, D], B16, tag="w1c", name="w1c")
                    nc.sync.dma_start(w1c[:], t_w1[l][ft])
                    ps = ps_tile([128, S])
                    for dc in range(DC):
                        nc.tensor.matmul(ps[:], w1c[:, dc * 128:(dc + 1) * 128],
                                         h2T[:, dc, :], start=(dc == 0),
                                         stop=(dc == DC - 1))
                    nc.scalar.activation(ffT[:, ft, :], ps[:], Act.Relu,
                                         bias=b1_sb[:, ft:ft + 1])

                # ---- FFN down + residual ----
                pss = [ps_tile([128, NSL]) for _ in range(2 * ST)]
                for fc in range(FT):
                    w2c = pw2.tile([128, D], B16, tag="w2c", name="w2c")
                    nc.sync.dma_start(w2c[:], t_w2[l][fc])
                    for st in range(ST):
                        for sl in range(2):
                            nc.tensor.matmul(
                                pss[st * 2 + sl][:], ffT[:, fc, st * 128:(st + 1) * 128],
                                w2c[:, sl * NSL:(sl + 1) * NSL],
                                start=(fc == 0), stop=False)
                for st in range(ST):
                    xi = ST * b + st
                    for sl in range(2):
                        nc.tensor.matmul(pss[st * 2 + sl][:], ones[:],
                                         br_sb[0:1, D + sl * NSL:D + (sl + 1) * NSL],
                                         start=False, stop=True)
                        nc.vector.tensor_add(x[:, xi, sl * NSL:(sl + 1) * NSL],
                                             x[:, xi, sl * NSL:(sl + 1) * NSL],
                                             pss[st * 2 + sl][:])


            for b in range(BL):
                attn_phase(b)
            for b in range(BL):
                ffn_phase(b)
        # ---- final LN + store ----
        g_f = bcast_rows(lnf_sb[0:1, 0:D])
        b_f = bcast_rows(lnf_sb[0:1, D:2 * D])
        for i in range(NT):
            ag, rstd = ln_stats(x[:, i, :])
            fo = pfin.tile([128, D], F32, tag="fo", name="fo")
            nc.vector.tensor_scalar(fo[:], x[:, i, :], ag[:, 0:1], rstd[:],
                                    op0=Alu.subtract, op1=Alu.mult)
            nc.vector.tensor_mul(fo[:], fo[:], g_f[:])
            nc.vector.tensor_add(fo[:], fo[:], b_f[:])
            nc.sync.dma_start(t_out[i], fo[:])


def prepare_inputs(inputs, n_layers=L):
    """Host-side shard/pack. Returns list of 8 per-core input maps."""
    f32 = np.float32
    idx = np.asarray(inputs["idx"]).astype(np.int32)           # [B, S]
    tok = np.ascontiguousarray(np.asarray(inputs["tok_emb"], dtype=f32).astype(BF16))
    seg_emb = np.asarray(inputs["seg_emb"], dtype=f32)
    pos = np.asarray(inputs["pos_emb"], dtype=f32)
    seg_pat = np.zeros(S, np.int64); seg_pat[S // 2 + 1:] = 1
    static = (pos[:S] + seg_emb[seg_pat]).astype(f32)          # [S, D]
    semb = np.ascontiguousarray(static.reshape(ST, 128, D).transpose(1, 0, 2))

    def rows2(g, b):
        return np.concatenate([np.asarray(g), np.asarray(b)])[None].astype(f32).astype(BF16)

    lnemb = rows2(inputs["ln_emb_g"], inputs["ln_emb_b"])
    lnf = rows2(inputs["lnf_g"], inputs["lnf_b"])

    sl = slice(0, n_layers)
    Wq = np.asarray(inputs["Wq"], dtype=f32)[sl]
    Wk = np.asarray(inputs["Wk"], dtype=f32)[sl]
    Wv = np.asarray(inputs["Wv"], dtype=f32)[sl]
    Wo = np.asarray(inputs["Wo"], dtype=f32)[sl]
    W1 = np.asarray(inputs["W1"], dtype=f32)[sl]
    W2 = np.asarray(inputs["W2"], dtype=f32)[sl]
    nl = n_layers

    def packw(w):  # [nl, D(d), D(m)] -> [nl, 128(p), DC(dc), D(m)] bf16
        return np.ascontiguousarray(
            w.reshape(nl, DC, 128, D).transpose(0, 2, 1, 3)).astype(BF16)

    wq = packw(Wq.transpose(0, 2, 1, 3).reshape(nl, D, D))
    wk = packw(Wk.transpose(0, 2, 1, 3).reshape(nl, D, D))
    wv = packw(Wv.transpose(0, 2, 1, 3).reshape(nl, D, D))
    wo = packw(Wo)
    w1p = np.ascontiguousarray(
        W1.reshape(nl, DC, 128, FT, 128).transpose(0, 3, 2, 1, 4)
        .reshape(nl, FT, 128, D)).astype(BF16)
    w2p = np.ascontiguousarray(W2.reshape(nl, FT, 128, D)).astype(BF16)

    lng = np.ascontiguousarray(np.concatenate([
        np.asarray(inputs["ln1_g"], dtype=f32)[sl].reshape(nl, DC, 128),
        np.asarray(inputs["ln2_g"], dtype=f32)[sl].reshape(nl, DC, 128)],
        axis=1).transpose(0, 2, 1))
    lnb = np.ascontiguousarray(np.concatenate([
        np.asarray(inputs["ln1_b"], dtype=f32)[sl].reshape(nl, DC, 128),
        np.asarray(inputs["ln2_b"], dtype=f32)[sl].reshape(nl, DC, 128)],
        axis=1).transpose(0, 2, 1))
    b1t = np.ascontiguousarray(
        np.asarray(inputs["b1"], dtype=f32)[sl].reshape(nl, FT, 128)
        .transpose(0, 2, 1))
    brows = np.concatenate([np.asarray(inputs["bo"], dtype=f32)[sl],
                            np.asarray(inputs["b2"], dtype=f32)[sl]],
                           axis=1)[:, None, :].astype(BF16)

    wb = np.empty(WB_TOTAL, BF16)
    def put_w(nm, arr):
        a = np.ascontiguousarray(arr).reshape(-1)
        wb[WB_OFF[nm]:WB_OFF[nm] + a.size] = a
    put_w("wq", wq); put_w("wk", wk); put_w("wv", wv); put_w("wo", wo)
    put_w("w1p", w1p); put_w("w2p", w2p); put_w("brows", brows)
    put_w("lnemb", lnemb); put_w("lnf", lnf)
    fb = np.empty(FB_TOTAL, np.float32)
    def put_f(nm, arr):
        a = np.ascontiguousarray(arr).reshape(-1)
        fb[FB_OFF[nm]:FB_OFF[nm] + a.size] = a
    put_f("semb", semb); put_f("lng", lng); put_f("lnb", lnb); put_f("b1t", b1t)

    shared = dict(temb=tok, wb=wb, fb=fb)
    in_maps = []
    for c in range(NCORES):
        flat = idx[BL * c:BL * (c + 1)].reshape(-1)            # [1024]
        idxw = np.ascontiguousarray(flat.reshape(NT, 128).T)   # [128, NT]
        in_maps.append(dict(idxw=idxw, **shared))
    return in_maps


def assemble_output(results):
    out = np.empty((B, S, D), np.float32)
    for c in range(NCORES):
        xo = results[c]["xo"]                                   # [NT, 128, D]
        for j in range(NT):
            out[BL * c + j // ST, (j % ST) * 128:(j % ST + 1) * 128, :] = xo[j]
    return out


_NC_CACHE = {}


def get_nc(n_layers=L):
    if n_layers not in _NC_CACHE:
        _NC_CACHE[n_layers] = build_nc(n_layers)
    return _NC_CACHE[n_layers]


def kernel(**inputs):
    nc = get_nc(L)
    in_maps = prepare_inputs(inputs, L)
    res = bass_utils.run_bass_kernel_spmd(nc, in_maps, core_ids=list(range(NCORES)))
    return assemble_output(res.results)

